# revision 17
# baseline (speedup 1.0000x reference)
"""Self-contained Trainium2 kernel for nn_AttentionHelper (B=16, C=128, L=2048).

reference:
    energy = einsum("bcq,bck->bqk", Q, K) / sqrt(C)
    att    = softmax(energy + log(mask + 1e-6), axis=-1) * mask
    out    = einsum("bck,bqk->bcq", V, att)
    return out, att.transpose(0, 2, 1)

Strategy (data-parallel over batch, 2 batches per core on 8 cores):
  - Q/K/V loaded fp32 (HWDGE) and cast to bf16 on-chip; PE streams bf16 at
    twice the fp32 rate, fp32 PSUM accumulation keeps the contraction
    accurate (~3e-3 overall, gate is 2e-2).
  - E^T = K^T Q computed directly in [k, q] layout (lhsT = K chunk, rhs = Q).
  - One ACT pass per chunk: P = exp(E^T/sqrt(C) + 2*log(m+eps)) with a
    per-partition bias AP (k is the partition axis in this layout). The
    doubled log folds the post-softmax multiplicative mask into the same
    pass: (m+eps)^2 = m*(m+eps) up to 1e-6 relative. P is stored bf16.
  - denom[q] = sum_k P via PE matmul with all-ones lhsT (M=1, col-tiled
    concurrent accumulators). Masked keys contribute ~1e-7 relative, so the
    exact 1/(m+eps) weighting of the reference denominator is unnecessary.
  - 1/denom is broadcast across partitions with a PE "select" matmul
    (lhsT = one-hot row matrix), which also discards the junk partitions.
  - A^T = P * (1/denom)[q]: DVE multiply (bf16, 2x mode) in place, then each
    chunk is DMA'd out (bf16 DRAM, cast back to f32 on the host).
  - out = V @ A^T accumulated over k chunks (lhsT = V^T via PE transposes).
  - 4-stage software pipeline at (batch, q-half) granularity: phase 2 of
    stage s overlaps phase 1 of stage s+1, which shortens the exposed
    startup and tail to half a batch each.
"""

import math
import sys
from contextlib import ExitStack

if "/opt/trn_rl_repo" not in sys.path:
    sys.path.insert(0, "/opt/trn_rl_repo")

import numpy as np

import concourse.bass as bass  # noqa: F401  (engine types referenced via nc)
import concourse.mybir as mybir
import concourse.tile as tile
from concourse import bacc
from concourse.bass_utils import run_bass_kernel_spmd
from concourse.masks import make_identity

B, C, L = 16, 128, 2048
N_CORES = 8
BPC = B // N_CORES  # batches per core
NK = L // 128  # 16 key chunks of 128
HALF = L // 2  # 1024, the q-half each pipeline stage covers
SCALE = 1.0 / math.sqrt(C)
EPS = 1e-6
AV_LAG = 3

F32 = mybir.dt.float32
BF16 = mybir.dt.bfloat16
AF = mybir.ActivationFunctionType

_CACHE: dict = {}
LAST_RESULTS = None


class _State:
    pass


def _build():
    nc = bacc.Bacc("TRN2", target_bir_lowering=False, num_devices=N_CORES)

    q_d = nc.declare_dram_parameter("q", [BPC, C, L], F32, isOutput=False)
    k_d = nc.declare_dram_parameter("k", [BPC, C, L], F32, isOutput=False)
    v_d = nc.declare_dram_parameter("v", [BPC, C, L], F32, isOutput=False)
    m_d = nc.declare_dram_parameter("mask", [BPC, 1, L], F32, isOutput=False)
    out_d = nc.declare_dram_parameter("out", [BPC, C, L], F32, isOutput=True)
    at_d = nc.declare_dram_parameter("attn_t", [BPC, L, L], BF16, isOutput=True)

    with tile.TileContext(nc) as tc, ExitStack() as ctx:
        sing = ctx.enter_context(tc.tile_pool(name="singles", bufs=1))
        qp = ctx.enter_context(tc.tile_pool(name="qp", bufs=2))
        kp = ctx.enter_context(tc.tile_pool(name="kp", bufs=2))
        vp = ctx.enter_context(tc.tile_pool(name="vp", bufs=2))
        vtp = ctx.enter_context(tc.tile_pool(name="vtp", bufs=2))
        rbp = ctx.enter_context(tc.tile_pool(name="rbp", bufs=2))
        outp = ctx.enter_context(tc.tile_pool(name="outp", bufs=2))
        maskp = ctx.enter_context(tc.tile_pool(name="maskp", bufs=2))
        stagep = ctx.enter_context(tc.tile_pool(name="stagep", bufs=2))
        p2tp = ctx.enter_context(tc.tile_pool(name="p2t", bufs=40))
        # PSUM budget (8 banks): et 2x[128,1024]=4 (also hosts V^T transposes
        # and the select-broadcast), dn 2x[128,512]=2, out 1x[128,1024]=2
        ps_et = ctx.enter_context(tc.tile_pool(name="ps_et", bufs=2, space="PSUM"))
        ps_dn = ctx.enter_context(tc.tile_pool(name="ps_dn", bufs=2, space="PSUM"))
        ps_out = ctx.enter_context(tc.tile_pool(name="ps_out", bufs=1, space="PSUM"))

        ident = sing.tile([128, 128], BF16)
        make_identity(nc, ident)
        ones_col = sing.tile([128, 1], BF16, tag="ones_col")
        nc.vector.memset(ones_col[:], 1.0)
        # sel[n][p, m] = 1 if p == 32n else 0 — "pick row 32n" via matmul
        sels = []
        for n in range(2):
            sel = sing.tile([128, 128], BF16, tag=f"sel{n}")
            nc.vector.memset(sel[:], 0.0)
            nc.vector.memset(sel[32 * n : 32 * n + 1, :], 1.0)
            sels.append(sel)

        bst = [_State() for _ in range(BPC)]  # per-batch
        sst = [_State() for _ in range(2 * BPC)]  # per-stage (b, h)

        def emit_inputs(b):
            s = bst[b]
            s.m_cols = maskp.tile([128, NK], F32, tag="m_cols")
            nc.sync.dma_start(
                s.m_cols[:], m_d.ap()[b].rearrange("o (j p) -> (o p) j", p=128)
            )
            # fp32 HWDGE loads into staging, then DVE cast to bf16
            for name, dram in (("k", k_d), ("q", q_d), ("v", v_d)):
                stg = stagep.tile([C, L], F32, tag="stg")
                nc.sync.dma_start(stg[:], dram.ap()[b])
                bf = {"q": qp, "k": kp, "v": vp}[name].tile([C, L], BF16)
                nc.vector.tensor_copy(bf[:], stg[:])
                setattr(s, name, bf)

        def emit_maskprep(b):
            s = bst[b]
            mpe = maskp.tile([128, NK], F32, tag="mpe")
            nc.vector.tensor_scalar_add(mpe[:], s.m_cols[:], EPS)
            s.logm2 = maskp.tile([128, NK], F32, tag="logm2")
            nc.scalar.activation(s.logm2[:], mpe[:], AF.Ln)
            nc.scalar.mul(s.logm2[:], s.logm2[:], 2.0)

        def emit_vt(b):
            # V^T (vt[p, 128*kb + c] = V[c, 128*kb + p]); psum via the et pool
            s = bst[b]
            s.vt = vtp.tile([128, L], BF16)
            for g in range(2):
                pvt = ps_et.tile([128, 1024], BF16, tag="et")
                for j in range(8):
                    kb = 8 * g + j
                    nc.tensor.transpose(
                        pvt[:, j * 128 : (j + 1) * 128],
                        s.v[:, kb * 128 : (kb + 1) * 128],
                        ident[:],
                    )
                nc.scalar.copy(s.vt[:, g * 1024 : (g + 1) * 1024], pvt[:])

        def stage(b, h):
            return sst[2 * b + h]

        def emit_ph1_start(b, h):
            s = stage(b, h)
            s.dn_ps = ps_dn.tile([128, 512], F32)
            # unused partitions must stay finite: 0 * inf = NaN would leak
            # through the select matmul in emit_boundary
            nc.vector.memset(s.dn_ps[:], 1.0)
            s.chunks = []

        def emit_et_exp(b, h, kb):
            s = stage(b, h)
            sb = bst[b]
            p2t = p2tp.tile([128, HALF], BF16)
            s.chunks.append(p2t)
            et = ps_et.tile([128, 1024], F32, tag="et")
            for n in range(2):
                nc.tensor.matmul(
                    et[:, n * 512 : (n + 1) * 512],
                    lhsT=sb.k[:, kb * 128 : (kb + 1) * 128],
                    rhs=sb.q[:, h * HALF + n * 512 : h * HALF + (n + 1) * 512],
                    start=True,
                    stop=True,
                )
            nc.scalar.activation(
                p2t[:],
                et[:],
                AF.Exp,
                bias=sb.logm2[:, kb : kb + 1],
                scale=SCALE,
            )

        def emit_dn(b, h, kb):
            s = stage(b, h)
            for n in range(2):
                nc.tensor.matmul(
                    s.dn_ps[32 * n : 32 * n + 1, :],
                    lhsT=ones_col[:],
                    rhs=s.chunks[kb][:, n * 512 : (n + 1) * 512],
                    start=(kb == 0),
                    stop=(kb == NK - 1),
                    tile_position=(0, 32 * n),
                )

        def emit_ph1_chunk(b, h, kb):
            emit_et_exp(b, h, kb)
            if kb >= 1:
                emit_dn(b, h, kb - 1)
            if kb == NK - 1:
                emit_dn(b, h, kb)

        def emit_boundary(b, h):
            # r = 1/denom; slices live at partitions 0/32 of dn_ps, junk on
            # the others is discarded by the select matmuls
            s = stage(b, h)
            rec32 = maskp.tile([128, 512], F32, tag="rec32")
            nc.vector.reciprocal(rec32[:], s.dn_ps[:])
            rec = maskp.tile([128, 512], BF16, tag="rec")
            nc.scalar.copy(rec[:], rec32[:])
            s.rbc = rbp.tile([128, HALF], BF16)
            rb_ps = ps_et.tile([128, 1024], F32, tag="et")
            for n in range(2):
                nc.tensor.matmul(
                    rb_ps[:, n * 512 : (n + 1) * 512],
                    lhsT=sels[n][:],
                    rhs=rec[:],
                    start=True,
                    stop=True,
                )
            nc.scalar.copy(s.rbc[:], rb_ps[:])

        def emit_rmul_dma(b, h, kb):
            s = stage(b, h)
            p2t = s.chunks[kb]
            nc.vector.tensor_mul(p2t[:], p2t[:], s.rbc[:])
            nc.sync.dma_start(
                at_d.ap()[b, kb * 128 : (kb + 1) * 128, h * HALF : (h + 1) * HALF],
                p2t[:],
            )

        def emit_av(b, h, kb):
            s = stage(b, h)
            if kb == 0:
                s.out_ps = ps_out.tile([128, 1024], F32, tag="out")
            for n in range(2):
                nc.tensor.matmul(
                    s.out_ps[:, n * 512 : (n + 1) * 512],
                    lhsT=bst[b].vt[:, kb * 128 : (kb + 1) * 128],
                    rhs=s.chunks[kb][:, n * 512 : (n + 1) * 512],
                    start=(kb == 0),
                    stop=(kb == NK - 1),
                )

        def emit_ph2_chunk(b, h, kb):
            emit_rmul_dma(b, h, kb)
            if kb >= AV_LAG:
                emit_av(b, h, kb - AV_LAG)
            if kb == NK - 1:
                for j in range(NK - AV_LAG, NK):
                    emit_av(b, h, j)

        def emit_out(b, h):
            s = stage(b, h)
            osb = outp.tile([128, HALF], F32)
            nc.scalar.copy(osb[:], s.out_ps[:])
            nc.sync.dma_start(out_d.ap()[b][:, h * HALF : (h + 1) * HALF], osb[:])

        stages = [(b, h) for b in range(BPC) for h in range(2)]

        # software-pipelined emission over 4 stages
        emit_inputs(0)
        emit_maskprep(0)
        emit_inputs(1)
        emit_ph1_start(0, 0)
        for kb in range(NK):
            emit_ph1_chunk(0, 0, kb)
            if kb == 2:
                emit_vt(0)
        for si in range(len(stages)):
            b, h = stages[si]
            emit_boundary(b, h)
            if si + 1 < len(stages):
                nb, nh = stages[si + 1]
                if (nb, nh) == (1, 0):
                    emit_maskprep(1)
                emit_ph1_start(nb, nh)
                for kb in range(NK):
                    emit_ph2_chunk(b, h, kb)
                    emit_ph1_chunk(nb, nh, kb)
                    if (nb, nh) == (1, 0) and kb == 2:
                        emit_vt(1)
            else:
                for kb in range(NK):
                    emit_ph2_chunk(b, h, kb)
            emit_out(b, h)

    nc.compile()
    return nc


def kernel(proj_query, proj_key, proj_val, padding_mask):
    global LAST_RESULTS
    if "nc" not in _CACHE:
        _CACHE["nc"] = _build()
    nc = _CACHE["nc"]

    proj_query = np.ascontiguousarray(np.asarray(proj_query, dtype=np.float32))
    proj_key = np.ascontiguousarray(np.asarray(proj_key, dtype=np.float32))
    proj_val = np.ascontiguousarray(np.asarray(proj_val, dtype=np.float32))
    padding_mask = np.ascontiguousarray(np.asarray(padding_mask, dtype=np.float32))

    in_maps = []
    for i in range(N_CORES):
        s = slice(i * BPC, (i + 1) * BPC)
        in_maps.append(
            {
                "q": proj_query[s],
                "k": proj_key[s],
                "v": proj_val[s],
                "mask": padding_mask[s],
            }
        )

    res = run_bass_kernel_spmd(nc, in_maps, list(range(N_CORES)))
    LAST_RESULTS = res

    out = np.concatenate([res.results[i]["out"] for i in range(N_CORES)], axis=0)
    attn_t = np.concatenate(
        [np.asarray(res.results[i]["attn_t"], dtype=np.float32) for i in range(N_CORES)],
        axis=0,
    )
    return out, attn_t


# revision 18
# speedup vs baseline: 1.0527x; 1.0527x over previous
"""Self-contained Trainium2 kernel for nn_AttentionHelper (B=16, C=128, L=2048).

reference:
    energy = einsum("bcq,bck->bqk", Q, K) / sqrt(C)
    att    = softmax(energy + log(mask + 1e-6), axis=-1) * mask
    out    = einsum("bck,bqk->bcq", V, att)
    return out, att.transpose(0, 2, 1)

Strategy (data-parallel over batch, 2 batches per core on 8 cores):
  - Q/K/V loaded fp32 (HWDGE) and cast to bf16 on-chip; PE streams bf16 at
    twice the fp32 rate, fp32 PSUM accumulation keeps the contraction
    accurate (~3e-3 overall, gate is 2e-2).
  - E^T = K^T Q computed directly in [k, q] layout (lhsT = K chunk, rhs = Q).
  - One ACT pass per chunk: P = exp(E^T/sqrt(C) + 2*log(m+eps)) with a
    per-partition bias AP (k is the partition axis in this layout). The
    doubled log folds the post-softmax multiplicative mask into the same
    pass: (m+eps)^2 = m*(m+eps) up to 1e-6 relative. P is stored bf16.
  - denom[q] = sum_k P via PE matmul with all-ones lhsT (M=1, col-tiled
    concurrent accumulators). Masked keys contribute ~1e-7 relative, so the
    exact 1/(m+eps) weighting of the reference denominator is unnecessary.
  - 1/denom is broadcast across partitions with a PE "select" matmul
    (lhsT = one-hot row matrix), which also discards the junk partitions.
  - A^T = P * (1/denom)[q]: DVE multiply (bf16, 2x mode) in place, then each
    chunk is DMA'd out (bf16 DRAM, cast back to f32 on the host).
  - out = V @ A^T accumulated over k chunks (lhsT = V^T via PE transposes).
  - 4-stage software pipeline at (batch, q-half) granularity: phase 2 of
    stage s overlaps phase 1 of stage s+1, which shortens the exposed
    startup and tail to half a batch each.
"""

import math
import sys
from contextlib import ExitStack

if "/opt/trn_rl_repo" not in sys.path:
    sys.path.insert(0, "/opt/trn_rl_repo")

import numpy as np

import concourse.bass as bass  # noqa: F401  (engine types referenced via nc)
import concourse.mybir as mybir
import concourse.tile as tile
from concourse import bacc
from concourse.bass_utils import run_bass_kernel_spmd
from concourse.masks import make_identity

B, C, L = 16, 128, 2048
N_CORES = 8
BPC = B // N_CORES  # batches per core
NK = L // 128  # 16 key chunks of 128
HALF = L // 2  # 1024, the q-half each pipeline stage covers
SCALE = 1.0 / math.sqrt(C)
EPS = 1e-6
AV_LAG = 3

F32 = mybir.dt.float32
BF16 = mybir.dt.bfloat16
AF = mybir.ActivationFunctionType

_CACHE: dict = {}
LAST_RESULTS = None


class _State:
    pass


def _build():
    nc = bacc.Bacc("TRN2", target_bir_lowering=False, num_devices=N_CORES)

    q_d = nc.declare_dram_parameter("q", [BPC, C, L], F32, isOutput=False)
    k_d = nc.declare_dram_parameter("k", [BPC, C, L], F32, isOutput=False)
    v_d = nc.declare_dram_parameter("v", [BPC, C, L], F32, isOutput=False)
    m_d = nc.declare_dram_parameter("mask", [BPC, 1, L], F32, isOutput=False)
    out_d = nc.declare_dram_parameter("out", [BPC, C, L], F32, isOutput=True)
    at_d = nc.declare_dram_parameter("attn_t", [BPC, L, L], BF16, isOutput=True)

    with tile.TileContext(nc) as tc, ExitStack() as ctx:
        sing = ctx.enter_context(tc.tile_pool(name="singles", bufs=1))
        qp = ctx.enter_context(tc.tile_pool(name="qp", bufs=2))
        kp = ctx.enter_context(tc.tile_pool(name="kp", bufs=2))
        vp = ctx.enter_context(tc.tile_pool(name="vp", bufs=2))
        vtp = ctx.enter_context(tc.tile_pool(name="vtp", bufs=2))
        rbp = ctx.enter_context(tc.tile_pool(name="rbp", bufs=2))
        outp = ctx.enter_context(tc.tile_pool(name="outp", bufs=2))
        maskp = ctx.enter_context(tc.tile_pool(name="maskp", bufs=2))
        stagep = ctx.enter_context(tc.tile_pool(name="stagep", bufs=2))
        p2tp = ctx.enter_context(tc.tile_pool(name="p2t", bufs=40))
        # PSUM budget (8 banks): et 2x[128,1024]=4 (also hosts V^T transposes
        # and the select-broadcast), dn 2x[128,512]=2, out 1x[128,1024]=2
        ps_et = ctx.enter_context(tc.tile_pool(name="ps_et", bufs=2, space="PSUM"))
        ps_dn = ctx.enter_context(tc.tile_pool(name="ps_dn", bufs=2, space="PSUM"))
        ps_out = ctx.enter_context(tc.tile_pool(name="ps_out", bufs=1, space="PSUM"))

        ident = sing.tile([128, 128], BF16)
        make_identity(nc, ident)
        ones_col = sing.tile([128, 1], BF16, tag="ones_col")
        nc.vector.memset(ones_col[:], 1.0)
        # sel[n][p, m] = 1 if p == 32n else 0 — "pick row 32n" via matmul
        sels = []
        for n in range(2):
            sel = sing.tile([128, 128], BF16, tag=f"sel{n}")
            nc.vector.memset(sel[:], 0.0)
            nc.vector.memset(sel[32 * n : 32 * n + 1, :], 1.0)
            sels.append(sel)

        bst = [_State() for _ in range(BPC)]  # per-batch
        sst = [_State() for _ in range(2 * BPC)]  # per-stage (b, h)

        def emit_inputs(b):
            s = bst[b]
            s.m_cols = maskp.tile([128, NK], F32, tag="m_cols")
            nc.sync.dma_start(
                s.m_cols[:], m_d.ap()[b].rearrange("o (j p) -> (o p) j", p=128)
            )
            # fp32 HWDGE loads into staging, then DVE cast to bf16
            for name, dram in (("k", k_d), ("q", q_d), ("v", v_d)):
                stg = stagep.tile([C, L], F32, tag="stg")
                nc.sync.dma_start(stg[:], dram.ap()[b])
                bf = {"q": qp, "k": kp, "v": vp}[name].tile([C, L], BF16)
                nc.vector.tensor_copy(bf[:], stg[:])
                setattr(s, name, bf)

        def emit_maskprep(b):
            s = bst[b]
            mpe = maskp.tile([128, NK], F32, tag="mpe")
            nc.vector.tensor_scalar_add(mpe[:], s.m_cols[:], EPS)
            s.logm2 = maskp.tile([128, NK], F32, tag="logm2")
            nc.scalar.activation(s.logm2[:], mpe[:], AF.Ln)
            nc.scalar.mul(s.logm2[:], s.logm2[:], 2.0)

        def emit_vt(b):
            # V^T (vt[p, 128*kb + c] = V[c, 128*kb + p]); psum via the et pool
            s = bst[b]
            s.vt = vtp.tile([128, L], BF16)
            for g in range(2):
                pvt = ps_et.tile([128, 1024], BF16, tag="et")
                for j in range(8):
                    kb = 8 * g + j
                    nc.tensor.transpose(
                        pvt[:, j * 128 : (j + 1) * 128],
                        s.v[:, kb * 128 : (kb + 1) * 128],
                        ident[:],
                    )
                nc.scalar.copy(s.vt[:, g * 1024 : (g + 1) * 1024], pvt[:])

        def stage(b, h):
            return sst[2 * b + h]

        def emit_ph1_start(b, h):
            s = stage(b, h)
            s.dn_ps = ps_dn.tile([128, 512], F32)
            # unused partitions must stay finite: 0 * inf = NaN would leak
            # through the select matmul in emit_boundary
            nc.vector.memset(s.dn_ps[:], 1.0)
            s.chunks = []

        def emit_et_exp(b, h, kb):
            s = stage(b, h)
            sb = bst[b]
            p2t = p2tp.tile([128, HALF], BF16)
            s.chunks.append(p2t)
            et = ps_et.tile([128, 1024], F32, tag="et")
            for n in range(2):
                nc.tensor.matmul(
                    et[:, n * 512 : (n + 1) * 512],
                    lhsT=sb.k[:, kb * 128 : (kb + 1) * 128],
                    rhs=sb.q[:, h * HALF + n * 512 : h * HALF + (n + 1) * 512],
                    start=True,
                    stop=True,
                )
            nc.scalar.activation(
                p2t[:],
                et[:],
                AF.Exp,
                bias=sb.logm2[:, kb : kb + 1],
                scale=SCALE,
            )

        def emit_dn(b, h, kb):
            s = stage(b, h)
            for n in range(2):
                nc.tensor.matmul(
                    s.dn_ps[32 * n : 32 * n + 1, :],
                    lhsT=ones_col[:],
                    rhs=s.chunks[kb][:, n * 512 : (n + 1) * 512],
                    start=(kb == 0),
                    stop=(kb == NK - 1),
                    tile_position=(0, 32 * n),
                )

        def emit_ph1_chunk(b, h, kb):
            emit_et_exp(b, h, kb)
            if kb >= 1:
                emit_dn(b, h, kb - 1)
            if kb == NK - 1:
                emit_dn(b, h, kb)

        def emit_boundary(b, h):
            # r = 1/denom; slices live at partitions 0/32 of dn_ps, junk on
            # the others is discarded by the select matmuls
            s = stage(b, h)
            rec32 = maskp.tile([128, 512], F32, tag="rec32")
            nc.vector.reciprocal(rec32[:], s.dn_ps[:])
            rec = maskp.tile([128, 512], BF16, tag="rec")
            nc.scalar.copy(rec[:], rec32[:])
            s.rbc = rbp.tile([128, HALF], BF16)
            rb_ps = ps_et.tile([128, 1024], F32, tag="et")
            for n in range(2):
                nc.tensor.matmul(
                    rb_ps[:, n * 512 : (n + 1) * 512],
                    lhsT=sels[n][:],
                    rhs=rec[:],
                    start=True,
                    stop=True,
                )
            nc.scalar.copy(s.rbc[:], rb_ps[:])

        def emit_rmul_dma(b, h, kb):
            s = stage(b, h)
            p2t = s.chunks[kb]
            nc.vector.tensor_mul(p2t[:], p2t[:], s.rbc[:])
            nc.sync.dma_start(
                at_d.ap()[b, kb * 128 : (kb + 1) * 128, h * HALF : (h + 1) * HALF],
                p2t[:],
            )

        def emit_av(b, h, kb):
            s = stage(b, h)
            if kb == 0:
                s.out_ps = ps_out.tile([128, 1024], F32, tag="out")
            for n in range(2):
                nc.tensor.matmul(
                    s.out_ps[:, n * 512 : (n + 1) * 512],
                    lhsT=bst[b].vt[:, kb * 128 : (kb + 1) * 128],
                    rhs=s.chunks[kb][:, n * 512 : (n + 1) * 512],
                    start=(kb == 0),
                    stop=(kb == NK - 1),
                )

        def emit_ph2_chunk(b, h, kb):
            emit_rmul_dma(b, h, kb)
            if kb >= AV_LAG:
                emit_av(b, h, kb - AV_LAG)
            if kb == NK - 1:
                for j in range(NK - AV_LAG, NK):
                    emit_av(b, h, j)

        def emit_out(b, h):
            s = stage(b, h)
            osb = outp.tile([128, HALF], F32)
            nc.scalar.copy(osb[:], s.out_ps[:])
            nc.sync.dma_start(out_d.ap()[b][:, h * HALF : (h + 1) * HALF], osb[:])

        stages = [(b, h) for b in range(BPC) for h in range(2)]
        ph1_ptr = [0] * len(stages)

        def advance_ph1(si, count):
            # emit up to `count` more phase-1 chunks of stage si
            if si >= len(stages):
                return
            b, h = stages[si]
            for _ in range(count):
                kb = ph1_ptr[si]
                if kb >= NK:
                    return
                if kb == 0:
                    if (b, h) == (1, 0):
                        emit_maskprep(1)
                    emit_ph1_start(b, h)
                emit_ph1_chunk(b, h, kb)
                if h == 0 and kb == 2:
                    emit_vt(b)
                ph1_ptr[si] += 1

        # software-pipelined emission over 4 (batch, q-half) stages with a
        # rolling phase-1 lookahead that keeps the PE fed across boundaries
        emit_inputs(0)
        emit_maskprep(0)
        emit_inputs(1)
        advance_ph1(0, NK)
        PRE = 3
        emit_boundary(*stages[0])
        advance_ph1(1, PRE)
        for si in range(len(stages)):
            b, h = stages[si]
            for kb in range(NK):
                emit_ph2_chunk(b, h, kb)
                advance_ph1(si + 1, 1)
            advance_ph1(si + 1, NK)  # finish any phase-1 remainder
            advance_ph1(si + 2, PRE)  # lookahead past the next boundary
            if si + 1 < len(stages):
                emit_boundary(*stages[si + 1])
            emit_out(b, h)

    nc.compile()
    return nc


def kernel(proj_query, proj_key, proj_val, padding_mask):
    global LAST_RESULTS
    if "nc" not in _CACHE:
        _CACHE["nc"] = _build()
    nc = _CACHE["nc"]

    proj_query = np.ascontiguousarray(np.asarray(proj_query, dtype=np.float32))
    proj_key = np.ascontiguousarray(np.asarray(proj_key, dtype=np.float32))
    proj_val = np.ascontiguousarray(np.asarray(proj_val, dtype=np.float32))
    padding_mask = np.ascontiguousarray(np.asarray(padding_mask, dtype=np.float32))

    in_maps = []
    for i in range(N_CORES):
        s = slice(i * BPC, (i + 1) * BPC)
        in_maps.append(
            {
                "q": proj_query[s],
                "k": proj_key[s],
                "v": proj_val[s],
                "mask": padding_mask[s],
            }
        )

    res = run_bass_kernel_spmd(nc, in_maps, list(range(N_CORES)))
    LAST_RESULTS = res

    out = np.concatenate([res.results[i]["out"] for i in range(N_CORES)], axis=0)
    attn_t = np.concatenate(
        [np.asarray(res.results[i]["attn_t"], dtype=np.float32) for i in range(N_CORES)],
        axis=0,
    )
    return out, attn_t
